# revision 1
# baseline (speedup 1.0000x reference)
"""Distributed Trainium2 Bass kernel for nn_CrossAttention.

Reference computation (per batch b):
    q = x @ Wq.T + bq          (N, C)       C = 1024, H = 16 heads, D = 64
    k = enc @ Wk.T + bk        (T, C)
    v = enc @ Wv.T + bv        (T, C)
    att = softmax(q.k / sqrt(D))   per head
    y = (att @ v) @ Wp.T + bp  (N, C)

Sharding (8 cores): core c = (batch b = c//2, head-group g = c%2).
Each core owns 8 heads (512 channels) of Q/K/V for one batch, computes
attention for those heads, and a *partial* output projection using the
512 matching columns of Wp.  Host sums the two partials per batch and
adds bp.  No inter-core communication.

Device-side layout is "feature on partitions" throughout:
    Q^T, K^T : (512, N)  channel-major (projection computes W @ X^T),
               stored bf16 for the scores matmuls.
    V        : (T, 512)  token-major bf16, with a ones column appended per
               head so the attn@V matmul also emits the softmax denominator.
    S^T = K Q^T : (T-block=128 partitions, n free) fp32 in PSUM,
               exp(scale*s) on ScalarE -> P^T bf16.
    attn@V   : out = V'.T @ P^T -> (65, n) fp32 = [y^T ; denom],
               accumulated per 4-t-block slab into SBUF tiles.
    out-proj : O^T = Wp_cols^T.T @ Y^T  (partial, summed on host).

Projections and the output projection run as float32r (full-speed fp32
path on the TRN2 PE for moving dim >= 256).  Host pre-transposes inputs
so the device never transposes anything.
"""

import numpy as np
from contextlib import ExitStack

# ---------------------------------------------------------------- constants
B, N, T, C, H = 4, 2048, 2048, 1024, 16
G = 2                      # head groups (cores per batch)
N_CORES = 8
D = C // H                 # 64 head dim
HL = H // G                # 8 heads per core
CL = HL * D                # 512 local channels per core

_COMPILED = {}             # (aug_x, aug_e) -> compiled Bacc


def build(aug_x: bool, aug_e: bool, num_devices: int = N_CORES,
          n=N, t=T, c=C, hl=HL, d=D, repeat=1, proj_dtype="f32r"):
    """Build + compile the per-core SPMD program.  Parameterized so tests
    can build small versions for CoreSim (requires t == n), and repeat>1
    duplicates the body for wall-clock timing calibration."""
    import concourse.mybir as mybir
    import concourse.tile as tile
    from concourse import bacc

    f32 = mybir.dt.float32
    bf16 = mybir.dt.bfloat16
    f32r = mybir.dt.float32r
    pdt = {"f32r": f32r, "bf16": bf16}[proj_dtype]
    EXP = mybir.ActivationFunctionType.Exp

    cl = hl * d
    dp1 = d + 1
    KC = c // 128                      # contraction chunks (proj)
    NCH = n // 512                     # n chunks of 512
    TB = t // 128                      # t blocks of 128
    MQ = cl // 128                     # q/k channel blocks (== head pairs)
    MO = c // 128                      # output channel blocks
    PAIRS = hl // 2
    assert TB == NCH * 4 and MQ == PAIRS
    scale = 1.0 / float(np.sqrt(d))

    xrows = c + (1 if aug_x else 0)
    erows = c + (1 if aug_e else 0)
    kq_chunks = [(i * 128, 128) for i in range(KC)] + ([(c, 1)] if aug_x else [])
    ke_chunks = [(i * 128, 128) for i in range(KC)] + ([(c, 1)] if aug_e else [])

    nc = bacc.Bacc("TRN2", target_bir_lowering=False, debug=False,
                   enable_asserts=False, num_devices=num_devices)

    xt = nc.dram_tensor("xt", (xrows, n), pdt, kind="ExternalInput").ap()
    et = nc.dram_tensor("et", (erows, t), pdt, kind="ExternalInput").ap()
    wqt = nc.dram_tensor("wqt", (xrows, cl), pdt, kind="ExternalInput").ap()
    wkt = nc.dram_tensor("wkt", (erows, cl), pdt, kind="ExternalInput").ap()
    wvt = nc.dram_tensor("wvt", (erows, cl), pdt, kind="ExternalInput").ap()
    wpt = nc.dram_tensor("wpt", (cl, c), pdt, kind="ExternalInput").ap()
    ot = nc.dram_tensor("ot", (c, n), f32, kind="ExternalOutput").ap()

    def emit_body(tc):
        with ExitStack() as ctx:
            persist = ctx.enter_context(tc.tile_pool(name="persist", bufs=1))
            psum = ctx.enter_context(tc.tile_pool(name="psum", bufs=2,
                                                  space="PSUM"))
            spool = ctx.enter_context(tc.tile_pool(name="satt", bufs=2))

            qt = [persist.tile([128, n], bf16, name=f"qt{m}", tag=f"qt{m}")
                  for m in range(MQ)]
            kt = [persist.tile([128, t], bf16, name=f"kt{m}", tag=f"kt{m}")
                  for m in range(MQ)]
            vv = [persist.tile([128, hl * dp1], bf16, name=f"vv{i}",
                               tag=f"vv{i}") for i in range(TB)]
            wpt_sb = [persist.tile([128, c], pdt, name=f"wp{p}",
                                   tag=f"wp{p}") for p in range(PAIRS)]
            ones1 = persist.tile([dp1, d], f32r, name="ones1", tag="ones1")
            ones1f = persist.tile([dp1, d], f32, name="ones1f", tag="ones1f")
            nc.vector.memset(ones1f[d:dp1, :], 1.0)
            nc.vector.tensor_copy(ones1[d:dp1, :], ones1f[d:dp1, :])

            # --------------------------------------- phase KV (+ weights)
            kv_ctx = ExitStack()
            wk_pool = kv_ctx.enter_context(tc.tile_pool(name="wkp", bufs=1))
            es_pool = kv_ctx.enter_context(
                tc.tile_pool(name="esl", bufs=len(ke_chunks)))
            wk_sb, wv_sb = [], []

            def kv_iter(nt):
                """K^T and V for t-blocks 4nt..4nt+3.  The first iteration
                interleaves the weight-chunk loads with the es loads so the
                first matmul isn't stuck behind bulk DMA."""
                es = []
                for ki, (off, sz) in enumerate(ke_chunks):
                    if nt == 0:
                        wkc = wk_pool.tile([sz, cl], pdt, name=f"wkc{ki}",
                                           tag=f"wkc{ki}")
                        nc.sync.dma_start(wkc, wkt[off:off + sz, :])
                        wk_sb.append(wkc)
                        wvc = wk_pool.tile([sz, cl], pdt, name=f"wvc{ki}",
                                           tag=f"wvc{ki}")
                        nc.sync.dma_start(wvc, wvt[off:off + sz, :])
                        wv_sb.append(wvc)
                    e = es_pool.tile([128, 512], pdt, name="es", tag="es")
                    nc.sync.dma_start(
                        e[:sz, :], et[off:off + sz, nt * 512:(nt + 1) * 512])
                    es.append(e)
                nk = len(ke_chunks)
                for mh in (range(0, MQ, 2) if MQ > 1 else [0]):
                    ms = [m for m in (mh, mh + 1) if m < MQ]
                    ps = [psum.tile([128, 512], f32, name=f"pk{m}", tag="pa")
                          for m in ms]
                    for ki, (off, sz) in enumerate(ke_chunks):
                        for j, m in enumerate(ms):
                            nc.tensor.matmul(
                                ps[j], wk_sb[ki][:, m * 128:(m + 1) * 128],
                                es[ki][:sz, :],
                                start=(ki == 0), stop=(ki == nk - 1))
                    for j, m in enumerate(ms):
                        nc.vector.tensor_copy(
                            kt[m][:, nt * 512:(nt + 1) * 512], ps[j])
                for th in (0, 2):
                    ps = [psum.tile([128, cl], f32, name=f"pv{tb}", tag="pa")
                          for tb in (th, th + 1)]
                    for ki, (off, sz) in enumerate(ke_chunks):
                        for j, tb in enumerate((th, th + 1)):
                            nc.tensor.matmul(
                                ps[j], es[ki][:sz, tb * 128:(tb + 1) * 128],
                                wv_sb[ki],
                                start=(ki == 0), stop=(ki == nk - 1))
                    for j, tb in enumerate((th, th + 1)):
                        ti = nt * 4 + tb
                        src = ps[j].rearrange("p (h e) -> p h e", h=hl)
                        dst = vv[ti].rearrange("p (h e) -> p h e", h=hl)
                        nc.vector.tensor_copy(dst[:, :, 0:d], src)
                        nc.vector.memset(dst[:, :, d:dp1], 1.0)

            # --------------------------------------- phase Q (+ weights)
            q_ctx = ExitStack()
            wq_pool = q_ctx.enter_context(tc.tile_pool(name="wqp", bufs=1))
            xs_pool = q_ctx.enter_context(
                tc.tile_pool(name="xsl", bufs=len(kq_chunks)))
            wq_sb = []

            def q_iter(nq):
                xs = []
                for ki, (off, sz) in enumerate(kq_chunks):
                    if nq == 0:
                        wqc = wq_pool.tile([sz, cl], pdt, name=f"wqc{ki}",
                                           tag=f"wqc{ki}")
                        nc.sync.dma_start(wqc, wqt[off:off + sz, :])
                        wq_sb.append(wqc)
                    x = xs_pool.tile([128, 512], pdt, name="xs", tag="xs")
                    nc.sync.dma_start(
                        x[:sz, :], xt[off:off + sz, nq * 512:(nq + 1) * 512])
                    xs.append(x)
                nk = len(kq_chunks)
                for mh in (range(0, MQ, 2) if MQ > 1 else [0]):
                    ms = [m for m in (mh, mh + 1) if m < MQ]
                    ps = [psum.tile([128, 512], f32, name=f"pq{m}", tag="pa")
                          for m in ms]
                    for ki, (off, sz) in enumerate(kq_chunks):
                        for j, m in enumerate(ms):
                            nc.tensor.matmul(
                                ps[j], wq_sb[ki][:, m * 128:(m + 1) * 128],
                                xs[ki][:sz, :],
                                start=(ki == 0), stop=(ki == nk - 1))
                    for j, m in enumerate(ms):
                        nc.vector.tensor_copy(
                            qt[m][:, nq * 512:(nq + 1) * 512], ps[j])

            # --------------------------------------- attention
            def att_pair(nq, p, av, trange, first_slab):
                """Scores + exp + attn@V for head pair p of n-chunk nq over
                the t-blocks in trange (a slab).  The slab's attn@V partial
                lives in a short-lived PSUM tile and folds into the SBUF
                accumulators av, so PSUM av slots never block on the
                normalize chain."""
                h0, h1 = 2 * p, 2 * p + 1
                trange = list(trange)
                avp = (psum.tile([dp1, 512], f32, name="avp0", tag="av0",
                                 bufs=1),
                       psum.tile([dp1, 512], f32, name="avp1", tag="av1",
                                 bufs=1))
                for ti in trange:
                    sc = psum.tile([128, 1024], f32, name="sc", tag="sc2")
                    nc.tensor.matmul(
                        sc[:, 0:512],
                        kt[p][0:64, ti * 128:(ti + 1) * 128],
                        qt[p][0:64, nq * 512:(nq + 1) * 512],
                        start=True, stop=True)
                    nc.tensor.matmul(
                        sc[:, 512:1024],
                        kt[p][64:128, ti * 128:(ti + 1) * 128],
                        qt[p][64:128, nq * 512:(nq + 1) * 512],
                        start=True, stop=True)
                    pt = spool.tile([128, 1024], bf16, name="pt", tag="pt",
                                    bufs=3)
                    nc.scalar.activation(pt, sc, EXP, scale=scale)
                    nc.tensor.matmul(
                        avp[0], vv[ti][:, h0 * dp1:(h0 + 1) * dp1],
                        pt[:, 0:512],
                        start=(ti == trange[0]), stop=(ti == trange[-1]))
                    nc.tensor.matmul(
                        avp[1], vv[ti][:, h1 * dp1:(h1 + 1) * dp1],
                        pt[:, 512:1024],
                        start=(ti == trange[0]), stop=(ti == trange[-1]))
                for j in range(2):
                    if first_slab:
                        nc.vector.tensor_copy(av[j], avp[j])
                    else:
                        nc.vector.tensor_add(av[j], av[j], avp[j])

            def att_recip(av):
                """Early half of normalize: DVE reciprocals of the softmax
                denominators.  Emitted right after the pair's attention so
                the result is long ready when the PE broadcast runs."""
                rcs = []
                for j in range(2):
                    rc = spool.tile([dp1, 512], f32r, name="rc", tag="rc",
                                    bufs=6)
                    with nc.allow_low_precision(reason="f32r == f32 bits"):
                        nc.vector.reciprocal(rc[d:dp1, :], av[j][d:dp1, :])
                    rcs.append(rc)
                return rcs

            def att_finish(rcs, av, ytp):
                """Late half: PE broadcast of 1/denom, then y^T = av * R."""
                for j in range(2):
                    Rp = psum.tile([64, 512], f32, name="Rp", tag="sc2")
                    nc.tensor.matmul(Rp, ones1[d:dp1, :], rcs[j][d:dp1, :],
                                     start=True, stop=True)
                    if j == 0:
                        nc.vector.tensor_mul(ytp[0:64, :], av[j][0:d, :], Rp)
                    else:
                        ytm = spool.tile([64, 512], pdt, name="ytm",
                                         tag="ytm")
                        nc.vector.tensor_mul(ytm, av[j][0:d, :], Rp)
                        nc.sync.dma_start(ytp[64:128, :], ytm)

            def out_proj(nq, yts):
                for m in range(MO):
                    po = psum.tile([128, 512], f32, name="po", tag="pa")
                    for p in range(PAIRS):
                        nc.tensor.matmul(
                            po, wpt_sb[p][:, m * 128:(m + 1) * 128], yts[p],
                            start=(p == 0), stop=(p == PAIRS - 1))
                    ob = spool.tile([128, 512], f32, name="ob", tag="ob",
                                    bufs=2)
                    nc.vector.tensor_copy(ob, po)
                    nc.sync.dma_start(ot[m * 128:(m + 1) * 128,
                                         nq * 512:(nq + 1) * 512], ob)

            # SBUF attn@V accumulators, shared across n-chunks.
            av_sb = [(persist.tile([dp1, 512], f32, name=f"avs{p}0",
                                   tag=f"avs{p}0"),
                      persist.tile([dp1, 512], f32, name=f"avs{p}1",
                                   tag=f"avs{p}1"))
                     for p in range(PAIRS)]

            # Program order hand-interleaves phases KV/Q with ALL of
            # n-chunk 0's attention so ScalarE (exp) starts as soon as the
            # first K/V slab lands.  Normalize is software-pipelined ~2
            # attention units behind the pair that produced it (reciprocal
            # emitted immediately, PE broadcast + muls later) so the PE
            # never stalls on the DVE chain.
            yts_by = {nqi: [] for nqi in range(NCH)}
            pending = []

            def flush_one():
                rcs, av, nq2, _p2 = pending.pop(0)
                ytp = spool.tile([128, 512], pdt, name="ytp", tag="ytp",
                                 bufs=PAIRS + 2)
                att_finish(rcs, av, ytp)
                yts_by[nq2].append(ytp)
                if len(yts_by[nq2]) == PAIRS:
                    out_proj(nq2, yts_by[nq2])

            for nt in range(NCH):
                kv_iter(nt)
                q_iter(nt)
                if nt == 0:
                    for p in range(PAIRS):
                        nc.sync.dma_start(wpt_sb[p],
                                          wpt[p * 128:(p + 1) * 128, :])
                for p in range(PAIRS):
                    att_pair(0, p, av_sb[p], range(nt * 4, nt * 4 + 4),
                             first_slab=(nt == 0))
                    if nt == NCH - 1:
                        pending.append((att_recip(av_sb[p]), av_sb[p], 0, p))
            q_ctx.close()
            kv_ctx.close()

            # Flush BEFORE each attention unit: stageB(nq, p) must be
            # emitted before att_pair(nq+1, p) overwrites av_sb[p], and
            # pipeline depth stays <= 2 so rc tiles bound.
            for nq in range(1, NCH):
                for p in range(PAIRS):
                    while (len(pending) > 2
                           or any(e[3] == p for e in pending)):
                        flush_one()
                    for si in range(TB // 4):
                        att_pair(nq, p, av_sb[p], range(si * 4, si * 4 + 4),
                                 first_slab=(si == 0))
                    pending.append((att_recip(av_sb[p]), av_sb[p], nq, p))
            while pending:
                flush_one()

    with tile.TileContext(nc) as tc:
        for _rep in range(repeat):
            emit_body(tc)

    nc.compile()
    return nc


def _get_compiled(aug_x: bool, aug_e: bool):
    key = (aug_x, aug_e)
    if key not in _COMPILED:
        _COMPILED[key] = build(aug_x, aug_e)
    return _COMPILED[key]


def shard_inputs(x, enc, Wq, bq, Wk, bk, Wv, bv, Wp, aug_x, aug_e,
                 g_groups=G, cl=CL, proj_dtype="f32r"):
    if proj_dtype == "bf16":
        import ml_dtypes
        npdt = ml_dtypes.bfloat16
    else:
        npdt = np.float32
    in_maps = []
    n_cores = x.shape[0] * g_groups
    onesN = np.ones((1, x.shape[1]), np.float32)
    onesT = np.ones((1, enc.shape[1]), np.float32)
    for core in range(n_cores):
        b, g = divmod(core, g_groups)
        sl = slice(g * cl, (g + 1) * cl)
        xtc = x[b].T
        etc = enc[b].T
        wqtc = Wq[sl, :].T
        wktc = Wk[sl, :].T
        wvtc = Wv[sl, :].T
        if aug_x:
            xtc = np.concatenate([xtc, onesN], axis=0)
            wqtc = np.concatenate([wqtc, bq[sl][None, :]], axis=0)
        if aug_e:
            etc = np.concatenate([etc, onesT], axis=0)
            wktc = np.concatenate([wktc, bk[sl][None, :]], axis=0)
            wvtc = np.concatenate([wvtc, bv[sl][None, :]], axis=0)
        in_maps.append({
            "xt": np.ascontiguousarray(xtc, npdt),
            "et": np.ascontiguousarray(etc, npdt),
            "wqt": np.ascontiguousarray(wqtc, npdt),
            "wkt": np.ascontiguousarray(wktc, npdt),
            "wvt": np.ascontiguousarray(wvtc, npdt),
            "wpt": np.ascontiguousarray(Wp[:, sl].T, npdt),
        })
    return in_maps


def run_spmd(in_maps, nc=None, aug_x=False, aug_e=False, **kw):
    from concourse import bass_utils
    if nc is None:
        nc = _get_compiled(aug_x, aug_e)
    return bass_utils.run_bass_kernel_spmd(
        nc, in_maps, core_ids=list(range(len(in_maps))), **kw)


def kernel(**inputs):
    x = np.asarray(inputs["x"], np.float32)
    enc = np.asarray(inputs["encoder_output"], np.float32)
    Wq = np.asarray(inputs["Wq"], np.float32)
    bq = np.asarray(inputs["bq"], np.float32)
    Wk = np.asarray(inputs["Wk"], np.float32)
    bk = np.asarray(inputs["bk"], np.float32)
    Wv = np.asarray(inputs["Wv"], np.float32)
    bv = np.asarray(inputs["bv"], np.float32)
    Wp = np.asarray(inputs["Wp"], np.float32)
    bp = np.asarray(inputs["bp"], np.float32)

    aug_x = bool(np.any(bq))
    aug_e = bool(np.any(bk)) or bool(np.any(bv))
    nc = _get_compiled(aug_x, aug_e)
    in_maps = shard_inputs(x, enc, Wq, bq, Wk, bk, Wv, bv, Wp, aug_x, aug_e)
    res = run_spmd(in_maps, nc=nc)
    y = np.empty((B, N, C), np.float32)
    for b in range(B):
        y[b] = (res.results[2 * b]["ot"] +
                res.results[2 * b + 1]["ot"]).T + bp[None, :]
    return y



# revision 13
# speedup vs baseline: 1.2997x; 1.2997x over previous
"""Distributed Trainium2 Bass kernel for nn_CrossAttention.

Reference computation (per batch b):
    q = x @ Wq.T + bq          (N, C)       C = 1024, H = 16 heads, D = 64
    k = enc @ Wk.T + bk        (T, C)
    v = enc @ Wv.T + bv        (T, C)
    att = softmax(q.k / sqrt(D))   per head
    y = (att @ v) @ Wp.T + bp  (N, C)

Sharding (8 cores): core c = (batch b = c//2, head-group g = c%2).
Each core owns 8 heads (512 channels) of Q/K/V for one batch, computes
attention for those heads, and a *partial* output projection using the
512 matching columns of Wp.  Host sums the two partials per batch and
adds bp.  No inter-core communication.

Pipeline design (per core):
  * Projections bf16 (full PE rate, half the DMA of f32).
  * Scores bf16: per (nq-chunk, head-pair, t-block): two (128,512)
    matmuls into a (128, 1024) PSUM tile.
  * exp on ScalarE: ONE activation per (128, 1024) sc tile (2 heads),
    bf16 output.  ScalarE is the throughput limit (~33.5M exp at
    ~1 elem/lane/cycle ~ 220 us + 352-cycle per-instr overhead), so the
    schedule is built to keep it fed from the first K-chunk on.
  * attn@V bf16 per t-block and head, PSUM-accumulated across a visit's
    t-blocks (start/stop chain) -- no per-slab DVE adds.  V carries a
    ones column so the same matmul emits the softmax denominator.
  * Triangular schedule: at KV-chunk nt, attention S-steps {2nt,2nt+1}
    run for every n-chunk <= nt, so ScalarE saturates during the
    projection phase; the remainder drains after KV.
  * Normalize: denominator rows DMA-gathered into an (8,512) tile, one
    batched DVE reciprocal, GpSimd partition_broadcast of each row,
    DVE multiply.  Out-proj per n-chunk over the pair-stacked y tiles.
"""

import numpy as np
from contextlib import ExitStack

# ---------------------------------------------------------------- constants
B, N, T, C, H = 4, 2048, 2048, 1024, 16
G = 2                      # head groups (cores per batch)
N_CORES = 8
D = C // H                 # 64 head dim
HL = H // G                # 8 heads per core
CL = HL * D                # 512 local channels per core

_COMPILED = {}             # (aug_x, aug_e) -> compiled Bacc


def build(aug_x: bool, aug_e: bool, num_devices: int = N_CORES,
          n=N, t=T, c=C, hl=HL, d=D, repeat=1):
    """Build + compile the per-core SPMD program.  Parameterized so tests
    can build small versions for CoreSim (requires t == n)."""
    import concourse.mybir as mybir
    import concourse.tile as tile
    from concourse import bacc

    f32 = mybir.dt.float32
    bf16 = mybir.dt.bfloat16
    f32r = mybir.dt.float32r
    EXP = mybir.ActivationFunctionType.Exp

    cl = hl * d
    dp1 = d + 1
    KC = c // 128                      # contraction chunks (proj)
    NCH = n // 512                     # n chunks of 512
    TB = t // 128                      # t blocks of 128
    MQ = cl // 128                     # q/k channel blocks (== head pairs)
    MO = c // 128                      # output channel blocks
    PAIRS = hl // 2
    assert TB == NCH * 4 and MQ == PAIRS
    scale = 1.0 / float(np.sqrt(d))

    xrows = c + (1 if aug_x else 0)
    erows = c + (1 if aug_e else 0)
    kq_chunks = [(i * 128, 128) for i in range(KC)] + ([(c, 1)] if aug_x else [])
    ke_chunks = [(i * 128, 128) for i in range(KC)] + ([(c, 1)] if aug_e else [])

    nc = bacc.Bacc("TRN2", target_bir_lowering=False, debug=False,
                   enable_asserts=False, num_devices=num_devices)

    xt = nc.dram_tensor("xt", (xrows, n), bf16, kind="ExternalInput").ap()
    et = nc.dram_tensor("et", (erows, t), bf16, kind="ExternalInput").ap()
    wqt = nc.dram_tensor("wqt", (xrows, cl), bf16, kind="ExternalInput").ap()
    wkt = nc.dram_tensor("wkt", (erows, cl), bf16, kind="ExternalInput").ap()
    wvt = nc.dram_tensor("wvt", (erows, cl), bf16, kind="ExternalInput").ap()
    wpt = nc.dram_tensor("wpt", (cl, c), bf16, kind="ExternalInput").ap()
    ot = nc.dram_tensor("ot", (c, n), f32, kind="ExternalOutput").ap()

    def emit_body(tc):
        with ExitStack() as ctx:
            persist = ctx.enter_context(tc.tile_pool(name="persist", bufs=1))
            psum = ctx.enter_context(tc.tile_pool(name="psum", bufs=2,
                                                  space="PSUM"))
            spool = ctx.enter_context(tc.tile_pool(name="satt", bufs=2))

            qt = [persist.tile([128, n], bf16, name=f"qt{m}", tag=f"qt{m}")
                  for m in range(MQ)]
            kt = [persist.tile([128, t], bf16, name=f"kt{m}", tag=f"kt{m}")
                  for m in range(MQ)]
            # V bf16: (128, head, d+1) per t-block, ones column appended
            vv = [persist.tile([128, hl * dp1], bf16, name=f"vv{i}",
                               tag=f"vv{i}") for i in range(TB)]
            wpt_sb = [persist.tile([128, c], bf16, name=f"wp{p}",
                                   tag=f"wp{p}") for p in range(PAIRS)]

            # warm the exp activation table while the first DMAs run
            wu = spool.tile([1, 8], f32, name="wu", tag="wu", bufs=1)
            nc.vector.memset(wu, 0.0)
            wub = spool.tile([1, 8], bf16, name="wub", tag="wub", bufs=1)
            nc.scalar.activation(wub, wu, EXP, scale=1.0)

            # --------------------------------------- phase KV (+ weights)
            kv_ctx = ExitStack()
            wk_pool = kv_ctx.enter_context(tc.tile_pool(name="wkp", bufs=1))
            es_pool = kv_ctx.enter_context(
                tc.tile_pool(name="esl", bufs=len(ke_chunks)))
            wk_sb, wv_sb = [], []

            def kv_iter(nt):
                """K^T and V for t-blocks 4nt..4nt+3.  The first iteration
                interleaves the weight-chunk loads with the es loads so the
                first matmul isn't stuck behind bulk DMA."""
                es = []
                for ki, (off, sz) in enumerate(ke_chunks):
                    if nt == 0:
                        wkc = wk_pool.tile([sz, cl], bf16, name=f"wkc{ki}",
                                           tag=f"wkc{ki}")
                        nc.sync.dma_start(wkc, wkt[off:off + sz, :])
                        wk_sb.append(wkc)
                        wvc = wk_pool.tile([sz, cl], bf16, name=f"wvc{ki}",
                                           tag=f"wvc{ki}")
                        nc.sync.dma_start(wvc, wvt[off:off + sz, :])
                        wv_sb.append(wvc)
                    e = es_pool.tile([128, 512], bf16, name="es", tag="es")
                    nc.sync.dma_start(
                        e[:sz, :], et[off:off + sz, nt * 512:(nt + 1) * 512])
                    es.append(e)
                nk = len(ke_chunks)
                for mh in (range(0, MQ, 2) if MQ > 1 else [0]):
                    ms = [m for m in (mh, mh + 1) if m < MQ]
                    ps = [psum.tile([128, 512], f32, name=f"pk{m}", tag="pa")
                          for m in ms]
                    for ki, (off, sz) in enumerate(ke_chunks):
                        for j, m in enumerate(ms):
                            nc.tensor.matmul(
                                ps[j], wk_sb[ki][:, m * 128:(m + 1) * 128],
                                es[ki][:sz, :],
                                start=(ki == 0), stop=(ki == nk - 1))
                    for j, m in enumerate(ms):
                        nc.vector.tensor_copy(
                            kt[m][:, nt * 512:(nt + 1) * 512], ps[j])
                for th in (0, 2):
                    ps = [psum.tile([128, cl], f32, name=f"pv{tb}", tag="pa")
                          for tb in (th, th + 1)]
                    for ki, (off, sz) in enumerate(ke_chunks):
                        for j, tb in enumerate((th, th + 1)):
                            nc.tensor.matmul(
                                ps[j], es[ki][:sz, tb * 128:(tb + 1) * 128],
                                wv_sb[ki],
                                start=(ki == 0), stop=(ki == nk - 1))
                    for j, tb in enumerate((th, th + 1)):
                        ti = nt * 4 + tb
                        src = ps[j].rearrange("p (h e) -> p h e", h=hl)
                        dst = vv[ti].rearrange("p (h e) -> p h e", h=hl)
                        nc.vector.tensor_copy(dst[:, :, 0:d], src)
                        nc.vector.memset(dst[:, :, d:dp1], 1.0)

            # --------------------------------------- phase Q (+ weights)
            q_ctx = ExitStack()
            wq_pool = q_ctx.enter_context(tc.tile_pool(name="wqp", bufs=1))
            xs_pool = q_ctx.enter_context(
                tc.tile_pool(name="xsl", bufs=len(kq_chunks)))
            wq_sb = []

            def q_iter(nq):
                xs = []
                for ki, (off, sz) in enumerate(kq_chunks):
                    if nq == 0:
                        wqc = wq_pool.tile([sz, cl], bf16, name=f"wqc{ki}",
                                           tag=f"wqc{ki}")
                        nc.sync.dma_start(wqc, wqt[off:off + sz, :])
                        wq_sb.append(wqc)
                    x = xs_pool.tile([128, 512], bf16, name="xs", tag="xs")
                    nc.sync.dma_start(
                        x[:sz, :], xt[off:off + sz, nq * 512:(nq + 1) * 512])
                    xs.append(x)
                nk = len(kq_chunks)
                for mh in (range(0, MQ, 2) if MQ > 1 else [0]):
                    ms = [m for m in (mh, mh + 1) if m < MQ]
                    ps = [psum.tile([128, 512], f32, name=f"pq{m}", tag="pa")
                          for m in ms]
                    for ki, (off, sz) in enumerate(kq_chunks):
                        for j, m in enumerate(ms):
                            nc.tensor.matmul(
                                ps[j], wq_sb[ki][:, m * 128:(m + 1) * 128],
                                xs[ki][:sz, :],
                                start=(ki == 0), stop=(ki == nk - 1))
                    for j, m in enumerate(ms):
                        nc.vector.tensor_copy(
                            qt[m][:, nq * 512:(nq + 1) * 512], ps[j])

            # --------------------------------------- attention
            pend_av = []               # skew queue of attn@V matmuls
            av_sb = {}                 # (nq, p) -> SBUF accumulator

            def emit_avmm(avp, p, ti, pt, ch_start, ch_stop):
                vvr = vv[ti].rearrange("p (h e) -> p h e", h=hl)
                for h2 in range(2):
                    hh = 2 * p + h2
                    nc.tensor.matmul(
                        avp[:, h2 * 512:(h2 + 1) * 512],
                        vvr[:, hh, :],
                        pt[:, h2 * 512:(h2 + 1) * 512],
                        start=ch_start, stop=ch_stop)

            def flush_av(n_keep=1):
                while len(pend_av) > n_keep:
                    args = pend_av.pop(0)
                    emit_avmm(*args)

            def sstep(nq, p, S, avp, ch_start, ch_stop):
                """Scores + exp for (nq, p, S); attn@V is queued and
                emitted with a 2-t-block skew so a pending exp never sits
                at the head of the PE queue in front of independent work."""
                for i in range(2):
                    ti = 2 * S + i
                    sc = psum.tile([128, 1024], f32, name="sc", tag="sc")
                    for h2 in range(2):
                        nc.tensor.matmul(
                            sc[:, h2 * 512:(h2 + 1) * 512],
                            kt[p][h2 * 64:(h2 + 1) * 64,
                                  ti * 128:(ti + 1) * 128],
                            qt[p][h2 * 64:(h2 + 1) * 64,
                                  nq * 512:(nq + 1) * 512],
                            start=True, stop=True)
                    pt = spool.tile([128, 1024], bf16, name="pt", tag="pt",
                                    bufs=6)
                    nc.scalar.activation(pt, sc, EXP, scale=scale)
                    pend_av.append((avp, p, ti, pt,
                                    ch_start and i == 0, ch_stop and i == 1))
                    flush_av(n_keep=2)

            def unit_visit(nq, p, s_list):
                """Attention S-steps for unit (nq, p): accumulate the
                t-superblocks in s_list into a PSUM chain, then fold into
                the unit's SBUF accumulator."""
                avp = psum.tile([dp1, 1024], f32, name="avp", tag="av",
                                bufs=1)
                for j, S in enumerate(s_list):
                    sstep(nq, p, S, avp, j == 0, j == len(s_list) - 1)
                flush_av(n_keep=0)
                u = (nq, p)
                if u not in av_sb:
                    av_sb[u] = spool.tile([dp1, 1024], f32, name="avs",
                                          tag="avs", bufs=NCH * PAIRS)
                    nc.vector.tensor_copy(av_sb[u], avp)
                else:
                    nc.vector.tensor_add(av_sb[u], av_sb[u], avp)

            def finish_nq(nq):
                """Normalize + out-projection for n-chunk nq (all PAIRS
                units complete)."""
                # batched reciprocal on contiguous partitions, then park
                # each pair's row at partition 32p so the GpSimd broadcast
                # below meets its start-partition rule
                st = spool.tile([PAIRS, 1024], f32, name="st", tag="st",
                                bufs=1)
                for p in range(PAIRS):
                    nc.sync.dma_start(
                        st[p:p + 1, :], av_sb[(nq, p)][d:dp1, :])
                st2 = spool.tile([PAIRS, 1024], bf16, name="st2",
                                 tag="st2", bufs=1)
                with nc.allow_low_precision(reason="~0.4% on 1/denom"):
                    nc.vector.reciprocal(st2, st)
                # HW partition_broadcast only sources partition 0: park
                # every pair's reciprocal row side by side on partition 0
                stb = spool.tile([1, PAIRS * 1024], bf16, name="stb",
                                 tag="stb", bufs=1)
                for p in range(PAIRS):
                    nc.sync.dma_start(stb[0:1, p * 1024:(p + 1) * 1024],
                                      st2[p:p + 1, :])
                yts = []
                for p in range(PAIRS):
                    ytp = spool.tile([128, 512], bf16, name="ytp", tag="ytp",
                                     bufs=PAIRS + 2)
                    for h2 in range(2):
                        rcb = spool.tile([d, 512], bf16, name="rcb",
                                         tag="rcb", bufs=2)
                        nc.gpsimd.partition_broadcast(
                            rcb,
                            stb[0:1, p * 1024 + h2 * 512:
                                p * 1024 + (h2 + 1) * 512])
                        if h2 == 0:
                            nc.vector.tensor_mul(
                                ytp[0:d, :], av_sb[(nq, p)][0:d, 0:512], rcb)
                        else:
                            ytm = spool.tile([d, 512], bf16, name="ytm",
                                             tag="ytm", bufs=2)
                            nc.vector.tensor_mul(
                                ytm, av_sb[(nq, p)][0:d, 512:1024], rcb)
                            nc.sync.dma_start(ytp[d:2 * d, :], ytm)
                    yts.append(ytp)
                for m in range(MO):
                    po = psum.tile([128, 512], f32, name="po", tag="pa")
                    for p in range(PAIRS):
                        nc.tensor.matmul(
                            po, wpt_sb[p][:, m * 128:(m + 1) * 128], yts[p],
                            start=(p == 0), stop=(p == PAIRS - 1))
                    ob = spool.tile([128, 512], f32, name="ob", tag="ob",
                                    bufs=2)
                    nc.vector.tensor_copy(ob, po)
                    nc.sync.dma_start(ot[m * 128:(m + 1) * 128,
                                         nq * 512:(nq + 1) * 512], ob)

            # Triangular schedule: at KV-chunk nt, run S-steps {2nt, 2nt+1}
            # for every n-chunk <= nt so ScalarE (exp) saturates during the
            # projection phase.  The missed S-steps (nq > 0) drain after KV,
            # with each n-chunk's normalize + out-proj as PE filler.
            for nt in range(NCH):
                kv_iter(nt)
                q_iter(nt)
                if nt == 0:
                    for p in range(PAIRS):
                        nc.sync.dma_start(wpt_sb[p],
                                          wpt[p * 128:(p + 1) * 128, :])
                for nqv in range(nt + 1):
                    for p in range(PAIRS):
                        unit_visit(nqv, p, [2 * nt, 2 * nt + 1])
            q_ctx.close()
            kv_ctx.close()

            finish_nq(0)
            for nq in range(1, NCH):
                for p in range(PAIRS):
                    unit_visit(nq, p, list(range(0, 2 * nq)))
                finish_nq(nq)

    with tile.TileContext(nc) as tc:
        for _rep in range(repeat):
            emit_body(tc)

    nc.compile()
    return nc


def _get_compiled(aug_x: bool, aug_e: bool):
    key = (aug_x, aug_e)
    if key not in _COMPILED:
        _COMPILED[key] = build(aug_x, aug_e)
    return _COMPILED[key]


def shard_inputs(x, enc, Wq, bq, Wk, bk, Wv, bv, Wp, aug_x, aug_e,
                 g_groups=G, cl=CL):
    import ml_dtypes
    npdt = ml_dtypes.bfloat16
    in_maps = []
    n_cores = x.shape[0] * g_groups
    onesN = np.ones((1, x.shape[1]), np.float32)
    onesT = np.ones((1, enc.shape[1]), np.float32)
    for core in range(n_cores):
        b, g = divmod(core, g_groups)
        sl = slice(g * cl, (g + 1) * cl)
        xtc = x[b].T
        etc = enc[b].T
        wqtc = Wq[sl, :].T
        wktc = Wk[sl, :].T
        wvtc = Wv[sl, :].T
        if aug_x:
            xtc = np.concatenate([xtc, onesN], axis=0)
            wqtc = np.concatenate([wqtc, bq[sl][None, :]], axis=0)
        if aug_e:
            etc = np.concatenate([etc, onesT], axis=0)
            wktc = np.concatenate([wktc, bk[sl][None, :]], axis=0)
            wvtc = np.concatenate([wvtc, bv[sl][None, :]], axis=0)
        in_maps.append({
            "xt": np.ascontiguousarray(xtc.astype(npdt)),
            "et": np.ascontiguousarray(etc.astype(npdt)),
            "wqt": np.ascontiguousarray(wqtc.astype(npdt)),
            "wkt": np.ascontiguousarray(wktc.astype(npdt)),
            "wvt": np.ascontiguousarray(wvtc.astype(npdt)),
            "wpt": np.ascontiguousarray(Wp[:, sl].T.astype(npdt)),
        })
    return in_maps


def run_spmd(in_maps, nc=None, aug_x=False, aug_e=False, **kw):
    from concourse import bass_utils
    if nc is None:
        nc = _get_compiled(aug_x, aug_e)
    return bass_utils.run_bass_kernel_spmd(
        nc, in_maps, core_ids=list(range(len(in_maps))), **kw)


def kernel(**inputs):
    x = np.asarray(inputs["x"], np.float32)
    enc = np.asarray(inputs["encoder_output"], np.float32)
    Wq = np.asarray(inputs["Wq"], np.float32)
    bq = np.asarray(inputs["bq"], np.float32)
    Wk = np.asarray(inputs["Wk"], np.float32)
    bk = np.asarray(inputs["bk"], np.float32)
    Wv = np.asarray(inputs["Wv"], np.float32)
    bv = np.asarray(inputs["bv"], np.float32)
    Wp = np.asarray(inputs["Wp"], np.float32)
    bp = np.asarray(inputs["bp"], np.float32)

    aug_x = bool(np.any(bq))
    aug_e = bool(np.any(bk)) or bool(np.any(bv))
    nc = _get_compiled(aug_x, aug_e)
    in_maps = shard_inputs(x, enc, Wq, bq, Wk, bk, Wv, bv, Wp, aug_x, aug_e)
    res = run_spmd(in_maps, nc=nc)
    y = np.empty((B, N, C), np.float32)
    for b in range(B):
        y[b] = (res.results[2 * b]["ot"] +
                res.results[2 * b + 1]["ot"]).T + bp[None, :]
    return y
